# revision 11
# baseline (speedup 1.0000x reference)
"""AWQ 4-bit quantized linear (x @ dequant(qweight)) on 8 NeuronCores.

Column-parallel tensor sharding: each core owns OUT_F/8 = 1376 output
columns; x is replicated (host-repacked to [macro, partition, k-block,
token] so every per-macro DMA is one contiguous piece per partition).

Layout trick: qweight int32 columns are viewed as int16 on host (pure
reinterpret), and the device unpacks 4 nibbles per int16 word with
CONTIGUOUS DVE writes into an "s-major" permuted column order
o' = s*344 + j  (original column o = 8*(j//2) + 4*(j%2) + s).
Scales are host-permuted to match; the device computes y in permuted
column order and the host un-permutes y at the end.

Per-core kernel:
  - prologue: unpack zero-points (4 DVE ops, int16 kept -- the dequant
    sub is int16-int16 -> fp16, no cast), park z rows in a DRAM scratch
    so per-group rows can be DMA-broadcast across partitions; scales
    rows broadcast straight from the sc input (no staging);
  - dequant all 32 k-blocks into resident SBUF W tiles: 4x DVE nibble
    unpack (int16), DVE subtract broadcast zeros (fp16 out), DVE
    multiply broadcast scales. mul emission is software-pipelined one
    k-block apart so dependent DVE ops never run back-to-back;
  - pass1: y partial over k-blocks 0..HK for all tokens, PSUM-
    accumulated; macro 0's two token-tiles are k-interleaved and
    emitted block-by-block right behind the dequant of that block, so
    the PE starts within ~2 us of the first W tile; spilled fp16 to a
    DRAM scratch;
  - pass2: accumulate k-blocks HK..32 in PSUM, DVE-fused
    (psum + spill) -> fp16 eviction, DMA out.
Queue assignment keeps every DMA class on its own ring: sync carries
qweight/zero-broadcast/spill-readback, scalar carries scale-broadcast
and y output, gpsimd carries bulk x loads (prefetched one macro ahead)
and pass-1 spill writes. Output gathered host-side (concat shards +
un-permute columns).
"""

import ctypes

import numpy as np

try:  # un-wedge a stale axon tunnel left by a previously killed run
    _axon = ctypes.CDLL("/opt/axon/libaxon_pjrt.so")
    _axon.axon_reset.restype = ctypes.c_int64
    _axon.axon_reset()
except OSError:
    pass

import concourse.mybir as mybir
import concourse.tile as tile
from concourse import bacc
from concourse._compat import axon_active
from concourse.tile_rust import add_dep_helper

FP16 = mybir.dt.float16
FP32 = mybir.dt.float32
I16 = mybir.dt.int16

P = 128
N_CORES = 8
IN_F = 4096
OUT_F = 11008
GROUP = 128            # quant group size == k-block size
NG = IN_F // GROUP     # 32 k-blocks
TOK = 2 * 2048         # tokens

OSH = OUT_F // N_CORES     # 1376 out columns per core
OPACK = OSH // 8           # 172 packed int32 columns per core
OP16 = OSH // 4            # 344 int16 words per core

SHIFT = mybir.AluOpType.logical_shift_right
AND = mybir.AluOpType.bitwise_and

# device column layout: o' = s*OP16 + j holds original column
# o = 8*(j//2) + 4*(j%2) + s
_J = np.arange(OP16)
IDX = np.concatenate([8 * (_J // 2) + 4 * (_J % 2) + s for s in range(4)])
INV = np.empty(OSH, dtype=np.int64)
INV[IDX] = np.arange(OSH)

CHUNKS = ((0, 512), (512, 512), (1024, OSH - 1024))


def build_program(tok=TOK, in_f=IN_F, osh=OSH, tok_macro=256, hk=5):
    """Emit the SPMD per-core program. Returns the compiled Bacc module."""
    ng = in_f // GROUP
    op16 = osh // 4
    assert tok % tok_macro == 0 and tok_macro % P == 0

    nc = bacc.Bacc("TRN2", target_bir_lowering=False, debug=not axon_active())
    n_macro = tok // tok_macro
    tt_per_macro = tok_macro // P
    n_units = n_macro * tt_per_macro

    xt = nc.declare_dram_parameter("xt", [n_macro, P, ng, tok_macro], FP16,
                                   isOutput=False)
    qw = nc.declare_dram_parameter("qw", [P, ng, op16], I16, isOutput=False)
    qz = nc.declare_dram_parameter("qz", [ng, op16], I16, isOutput=False)
    sc = nc.declare_dram_parameter("sc", [ng, osh], FP16, isOutput=False)
    y = nc.declare_dram_parameter("y", [tok, osh], FP16, isOutput=True)
    zscr = nc.dram_tensor("zscratch", [ng, osh], I16)
    ysp = nc.dram_tensor("yspill", [tok, osh], FP16)

    with tile.TileContext(nc) as tc:
        with (
            tc.tile_pool(name="prpool", bufs=1) as prpool,
            tc.tile_pool(name="wpool", bufs=1) as wpool,
            tc.tile_pool(name="xpool", bufs=3) as xpool,
            tc.tile_pool(name="x2pool", bufs=2) as x2pool,
            tc.tile_pool(name="qwpool", bufs=2) as qwpool,
            tc.tile_pool(name="zbpool", bufs=3) as zbpool,
            tc.tile_pool(name="sbpool", bufs=3) as sbpool,
            tc.tile_pool(name="ipool", bufs=3) as ipool,
            tc.tile_pool(name="fpool", bufs=4) as fpool,
            tc.tile_pool(name="stpool", bufs=4) as stpool,
            tc.tile_pool(name="y1pool", bufs=2) as y1pool,
            tc.tile_pool(name="pspool", bufs=2, space="PSUM") as pspool,
        ):
            wts = []
            ywrites = {}

            def load_x(pool, m, k0, k1, tag, pace_on=None):
                # one DMA for blocks [k0,k1) of macro m; per-partition
                # contiguous thanks to the host [m, p, a, t] layout
                nk = k1 - k0
                xtile = pool.tile([P, nk * tok_macro], FP16, tag=tag,
                                  name=f"{tag}m{m}")
                post = nc.gpsimd.dma_start(
                    xtile[:].rearrange("p (a t) -> p a t", a=nk),
                    xt[m, :, k0:k1, :])
                if pace_on is not None:
                    add_dep_helper(post.ins, pace_on.ins, sync=True,
                                   reason="dma pacing")
                return xtile

            def emit_unit_mms(xtile, tt, ps, k0, k1, kbase, gk0=None, gk1=None):
                # [k0,k1) emitted now; [gk0,gk1) is the full accumulation
                # group (differs when a unit is emitted block-by-block)
                gk0 = k0 if gk0 is None else gk0
                gk1 = k1 if gk1 is None else gk1
                for k in range(k0, k1):
                    a = k - kbase
                    lhs = xtile[:, a * tok_macro + tt * P:
                                a * tok_macro + (tt + 1) * P]
                    for o0, on in CHUNKS:
                        nc.tensor.matmul(
                            ps[:, o0:o0 + on], lhs, wts[k][:, o0:o0 + on],
                            start=(k == gk0), stop=(k == gk1 - 1))

            def evict_spill(ps, r0):
                s16 = stpool.tile([P, osh], FP16, tag="s16")
                for o0, on in CHUNKS:
                    nc.scalar.copy(s16[:, o0:o0 + on], ps[:, o0:o0 + on])
                ywrites[r0] = nc.gpsimd.dma_start(ysp[r0:r0 + P, :], s16[:])

            def pass1_units(pre):
                # macros 1.. (macro 0 is emitted inline with the dequant
                # loop, k-interleaved across its two token-tiles)
                for m in range(1, n_macro):
                    xtile = pre.pop(m, None)
                    if xtile is None:
                        xtile = load_x(xpool, m, 0, hk, "xp1")
                    if m + 1 < n_macro and m + 1 not in pre:
                        pre[m + 1] = load_x(xpool, m + 1, 0, hk, "xp1")
                    for tt in range(tt_per_macro):
                        ps = pspool.tile([P, 1536], FP32, tag="ps", name="ps")
                        emit_unit_mms(xtile, tt, ps, 0, hk, 0)
                        evict_spill(ps, m * tok_macro + tt * P)
                        yield

            # ---- prologue: strict critical-path-first DMA order ----
            QCH = 8   # qweight k-blocks per chunked load

            def load_qw(c):
                qwt = qwpool.tile([P, QCH * op16], I16, tag="qw")
                nc.sync.dma_start(
                    qwt[:].rearrange("p (c o) -> p c o", c=QCH),
                    qw[:, c * QCH:(c + 1) * QCH, :])
                return qwt

            qzt = prpool.tile([ng, op16], I16)
            nc.sync.dma_start(qzt[:], qz[:])
            qwch = {0: load_qw(0)}
            pre = {0: load_x(xpool, 0, 0, hk, "xp1")}
            z16i = prpool.tile([ng, osh], I16)
            zunps = []
            for s in range(4):
                zunps.append(
                    nc.vector.tensor_scalar(z16i[:, s * op16:(s + 1) * op16],
                                            qzt[:], 4 * s, 15, SHIFT, AND))
            zwrite = nc.sync.dma_start(zscr[:], z16i[:])

            def load_zb(g, pace_on=None):
                zb = zbpool.tile([P, osh], I16, tag="zb")
                zbread = nc.sync.dma_start(
                    zb[:], zscr[g:g + 1, :].to_broadcast((P, osh)))
                add_dep_helper(zbread.ins, zwrite.ins, sync=True,
                               reason="zscr RAW")
                if pace_on is not None:
                    add_dep_helper(zbread.ins, pace_on.ins, sync=True,
                                   reason="dma pacing")
                return zb

            def load_sb(g, pace_on=None):
                # sync ring, NOT scalar: on the scalar ring these posts
                # would sit behind PSUM-evict ACTIVATEs in the ACT engine
                # FIFO and stall the dequant muls for tens of us
                sb = sbpool.tile([P, osh], FP16, tag="sb")
                sbread = nc.sync.dma_start(
                    sb[:], sc[g:g + 1, :].to_broadcast((P, osh)))
                if pace_on is not None:
                    add_dep_helper(sbread.ins, pace_on.ins, sync=True,
                                   reason="dma pacing")
                return sb

            # block 0's broadcasts get the fabric to themselves; later
            # prefetches are held back behind progress markers so the DMA
            # engines' fair-share never starves the oldest transfer
            zbpre = {0: load_zb(0)}
            sbpre = {0: load_sb(0)}
            zbpre[1] = load_zb(1, pace_on=zunps[3])
            sbpre[1] = load_sb(1, pace_on=zunps[3])
            pre[1] = load_x(xpool, 1, 0, hk, "xp1", pace_on=zunps[1])

            p1 = pass1_units(pre)
            emitted = 0
            pend = None  # (tmp, sb, g) whose mul is not yet emitted
            ps0 = [None, None]  # macro-0 psum tiles

            # ---- dequant all k-blocks, matmuls interleaved ----
            for g in range(ng):
                c, j = divmod(g, QCH)
                if j == 4 and c + 1 < ng // QCH:
                    qwch[c + 1] = load_qw(c + 1)   # prefetch next chunk
                qwt = qwch[c]
                if j == QCH - 1:
                    del qwch[c]
                zb = zbpre.pop(g, None)
                if zb is None:
                    zb = load_zb(g)
                sb = sbpre.pop(g, None)
                if sb is None:
                    sb = load_sb(g)

                qsl = qwt[:, j * op16:(j + 1) * op16]
                iw16i = ipool.tile([P, osh], I16, tag="iw16i")
                for s in range(4):
                    nc.vector.tensor_scalar(iw16i[:, s * op16:(s + 1) * op16],
                                            qsl, 4 * s, 15, SHIFT, AND)

                # emit the pending mul of k-block g-1 between this block's
                # unpack and sub so dependent DVE ops never run adjacent
                if pend is not None:
                    ptmp, psb, pg = pend
                    wt = wpool.tile([P, osh], FP16, tag=f"w{pg}",
                                    name=f"w{pg}")
                    nc.vector.tensor_mul(wt[:], ptmp[:], psb[:])
                    wts.append(wt)
                # int16 nibbles minus int16 zeros -> fp16
                tmp = fpool.tile([P, osh], FP16, tag="tmp")
                nc.vector.tensor_sub(tmp[:], iw16i[:], zb[:])
                pend = (tmp, sb, g)

                # macro 0, k-block g-1: both token-tiles, right behind
                # the mul that produced wts[g-1]
                if 1 <= g <= hk:
                    k = g - 1
                    for tt in range(tt_per_macro):
                        if ps0[tt] is None:
                            ps0[tt] = pspool.tile([P, 1536], FP32, tag="ps",
                                                  name="ps")
                        emit_unit_mms(pre[0], tt, ps0[tt], k, k + 1, 0,
                                      gk0=0, gk1=hk)
                    if k == hk - 1:
                        for tt in range(tt_per_macro):
                            evict_spill(ps0[tt], tt * P)

                if g > hk:
                    want = max(0, min((g - hk) * (n_units - 2) // (ng - hk - 6),
                                      n_units - 2))
                    while emitted < want:
                        next(p1)
                        emitted += 1
            ptmp, psb, pg = pend
            wt = wpool.tile([P, osh], FP16, tag=f"w{pg}", name=f"w{pg}")
            nc.vector.tensor_mul(wt[:], ptmp[:], psb[:])
            wts.append(wt)
            for _ in p1:
                emitted += 1

            # ---- pass 2: accumulate k>=hk, add spill, emit y ----
            x2pre = {hk_m: load_x(x2pool, hk_m, hk, ng, "xp2")
                     for hk_m in range(1)}
            for m in range(n_macro):
                xtile = x2pre.pop(m)
                if m + 1 < n_macro:
                    x2pre[m + 1] = load_x(x2pool, m + 1, hk, ng, "xp2")
                for tt in range(tt_per_macro):
                    r0 = m * tok_macro + tt * P
                    y1t = y1pool.tile([P, osh], FP16, tag="y1")
                    yread = nc.sync.dma_start(y1t[:], ysp[r0:r0 + P, :])
                    add_dep_helper(yread.ins, ywrites[r0].ins, sync=True,
                                   reason="yspill RAW")
                    ps = pspool.tile([P, 1536], FP32, tag="ps", name="ps")
                    emit_unit_mms(xtile, tt, ps, hk, ng, hk)
                    st = stpool.tile([P, osh], FP16, tag="st")
                    for o0, on in CHUNKS:
                        nc.vector.tensor_add(st[:, o0:o0 + on],
                                             ps[:, o0:o0 + on],
                                             y1t[:, o0:o0 + on])
                        nc.scalar.dma_start(y[r0:r0 + P, o0:o0 + on],
                                            st[:, o0:o0 + on])

    nc.compile()
    return nc


_PROGRAM = None

# test-harness hooks (unused by the grading path)
TRACE = False
TRACE_KWARGS = {}
LAST_RESULT = None


def _get_program():
    global _PROGRAM
    if _PROGRAM is None:
        _PROGRAM = build_program()
    return _PROGRAM


def kernel(x, qweight, qzeros, scales):
    from concourse.bass_utils import run_bass_kernel_spmd

    x = np.asarray(x)
    qweight = np.asarray(qweight)
    qzeros = np.asarray(qzeros)
    scales = np.asarray(scales)

    tok_macro = 256
    n_macro = TOK // tok_macro
    # [macro, partition, k-block, token]: per-(macro, partition) the
    # (block, token) plane is contiguous, so each macro DMA is a single
    # contiguous piece per partition
    xt = np.ascontiguousarray(
        x.reshape(n_macro, tok_macro, NG, P).transpose(0, 3, 2, 1))
    in_maps = []
    for c in range(N_CORES):
        qw_c = np.ascontiguousarray(
            qweight[:, c * OPACK:(c + 1) * OPACK]).view(np.int16)
        qw_c = np.ascontiguousarray(
            qw_c.reshape(NG, P, OP16).transpose(1, 0, 2))
        qz_c = np.ascontiguousarray(
            qzeros[:, c * OPACK:(c + 1) * OPACK]).view(np.int16)
        sc_c = np.ascontiguousarray(
            scales[:, c * OSH:(c + 1) * OSH][:, IDX])
        in_maps.append({"xt": xt, "qw": qw_c, "qz": qz_c, "sc": sc_c})

    nc = _get_program()
    res = run_bass_kernel_spmd(nc, in_maps, list(range(N_CORES)),
                               trace=TRACE, **TRACE_KWARGS)
    global LAST_RESULT
    LAST_RESULT = res
    yout = np.concatenate(
        [res.results[i]["y"][:, INV] for i in range(N_CORES)], axis=1)
    return yout.reshape(x.shape[0], x.shape[1], OUT_F)
